# revision 45
# baseline (speedup 1.0000x reference)
"""GATv2 (2-layer + linear head) Trainium2 Bass kernel, 8-core SPMD. v2.

Strategy vs v1: everything bf16 in the edge phase; only ONE per-edge DMA
gather (xl rows, 256B each) -- xr alignment is done on the tensor engine via
a transposed one-hot matmul whose dst-broadcast comes from a rank-1 PE
matmul; one-hot builds use full-rate tensor_tensor with stride-0 broadcast
APs (the v1 tensor_scalar-with-vector-scalars path measured 1.3us per
group); nodes are relabeled by descending in-degree on the host so windows
have uniform degree (less padding, uniform SPMD structure), with windows
round-robined across devices for load balance; xr table lives in SBUF.
Segment softmax uses unshifted exp (shift-invariant, safe range here).
"""
import sys
sys.path.insert(0, '/opt/trn_rl_repo')
import numpy as np
import ml_dtypes

BF16 = ml_dtypes.bfloat16

P = 128
N = 100000
NPAD = 100096             # 782 windows of 128
F = 128
H1 = 64
H2 = 32
NDEV = 8
NW = 98                   # windows per device (devices 6,7: 97 real + 1 dummy)
DNP = NW * P              # 12544
CHUNK = 32768
NCHUNK = (NPAD + CHUNK - 1) // CHUNK   # 4
MAXG = 8                  # groups (128 edges) per dma_gather
BW = 2                    # windows per batch (acc ring = 2*BW -> 2 batches in flight)
B4 = 4                    # groups per PSUM block in the edge inner loop
NB = 8                    # node tiles per batched DMA
ROW = 128                 # table row elems (bf16) = 256B stride


def _batches():
    out = []
    w = 0
    while w < NW:
        k = min(BW, NW - w)
        out.append((w, k))
        w += k
    return out


def _pack_idx16(idx):
    """idx: int array, len multiple of 128 -> [128, len//16] int16 tile data.
    Logical position i lives at [i % 16, i // 16], replicated over the 8
    16-partition groups."""
    n = len(idx)
    a = np.asarray(idx, np.int16).reshape(n // 16, 16).T
    return np.tile(a, (8, 1))


def _build_plan(src, dst):
    """src/dst int64 in PERMUTED node space. Returns uniform per-device
    layout (padded to cross-device maxima) plus per-device tables."""
    batches = _batches()
    NBT = len(batches)
    win = dst // P
    dev = win % NDEV
    wloc = win // NDEV
    jloc = dst % P
    ck = src // CHUNK

    w2b = np.zeros(NW, np.int64)
    w2i = np.zeros(NW, np.int64)
    for bi, (w0, k) in enumerate(batches):
        w2b[w0:w0 + k] = bi
        w2i[w0:w0 + k] = np.arange(k)

    counts = np.zeros((NDEV, NBT, NCHUNK, BW), np.int64)
    dev_edges = []
    for d in range(NDEV):
        m = dev == d
        s = src[m]
        j = jloc[m]
        wl = wloc[m]
        c = ck[m]
        bi = w2b[wl]
        wi = w2i[wl]
        key = (bi * NCHUNK + c) * BW + wi
        np.add.at(counts[d].reshape(-1), key, 1)
        order = np.lexsort((j, key))
        dev_edges.append((s[order], j[order], key[order]))

    gu = (counts.max(axis=0) + P - 1) // P       # [NBT, NCHUNK, BW]
    layout = []
    icol_off = 0
    gcol_off = 0
    for bi, (w0, k) in enumerate(batches):
        gp = 0
        instrs = []
        groups_w = []
        for c in range(NCHUNK):
            run = 0
            for w in range(BW):
                run += gu[bi, c, w]
                groups_w += [w] * int(gu[bi, c, w])
            a = 0
            while a < run:
                g = min(MAXG, run - a)
                instrs.append((c, gp + a, g))
                a += g
            gp += run
        icols = sum(8 * g for (_, _, g) in instrs)
        layout.append(dict(bi=bi, w0=w0, nw=k, GP=gp, instrs=instrs,
                           groups_w=groups_w, icol_off=icol_off,
                           gcol_off=gcol_off))
        icol_off += icols
        gcol_off += gp
    ICT, GCT = icol_off, gcol_off

    idx_all = np.zeros((NDEV, 128, ICT), np.int16)
    dstl_all = np.full((NDEV, 128, GCT), -1.0, ml_dtypes.bfloat16)
    dstlrow_all = np.full((NDEV, 1, GCT * P), -1.0, ml_dtypes.bfloat16)
    gu_flat = gu.reshape(-1)
    base_of_key = np.zeros(gu_flat.size + 1, np.int64)
    base_of_key[1:] = np.cumsum(gu_flat * P)
    gtot = int(gu.sum())
    for d in range(NDEV):
        s, j, key = dev_edges[d]
        kchange = np.r_[True, key[1:] != key[:-1]]
        runstart = np.maximum.accumulate(
            np.where(kchange, np.arange(len(key)), 0))
        within = np.arange(len(key)) - runstart
        slot = base_of_key[key] + within
        E_pad = gtot * P
        xl_rel = np.zeros(E_pad, np.int64)
        dstl_v = np.full(E_pad, -1.0, np.float32)
        xl_rel[slot] = s - (s // CHUNK) * CHUNK
        dstl_v[slot] = j
        dg = dstl_v.reshape(gtot, P)          # [group, e-lane]
        dstl_all[d] = dg.T.astype(ml_dtypes.bfloat16)
        dstlrow_all[d, 0] = dg.reshape(-1).astype(ml_dtypes.bfloat16)
        real = dstl_v >= 0
        for L in layout:
            bi = L["bi"]
            e0 = base_of_key[(bi * NCHUNK) * BW]
            ic = L["icol_off"]
            for (c, goff, G) in L["instrs"]:
                a0 = e0 + goff * P
                a1 = a0 + G * P
                idx_all[d, :, ic:ic + 8 * G] = _pack_idx16(xl_rel[a0:a1])
                ic += 8 * G
    return layout, ICT, GCT, idx_all, dstl_all, dstlrow_all


def _build_gat_layer(Cin, Cout, layout, ICT, GCT, final_linear):
    import concourse.bacc as bacc
    import concourse.mybir as mybir
    import concourse.tile as tile
    from concourse import library_config

    f32 = mybir.dt.float32
    bf16 = mybir.dt.bfloat16
    i16 = mybir.dt.int16
    AL = mybir.AluOpType
    AF = mybir.ActivationFunctionType

    nc = bacc.Bacc("TRN2", target_bir_lowering=False, debug=False,
                   num_swdge_queues=4, dynamic_dma_scratch_size=32768)
    t_xT = nc.dram_tensor("xT", [Cin, NPAD], bf16, kind="ExternalInput")
    t_xdT = nc.dram_tensor("xdT", [Cin, DNP], bf16, kind="ExternalInput")
    t_wl = nc.dram_tensor("wl", [Cin, Cout + 1], bf16, kind="ExternalInput")
    t_bl = nc.dram_tensor("bl", [128, Cout + 1], bf16, kind="ExternalInput")
    t_wr = nc.dram_tensor("wr", [Cin, Cout], bf16, kind="ExternalInput")
    t_br = nc.dram_tensor("br", [128, Cout], bf16, kind="ExternalInput")
    t_attb = nc.dram_tensor("attb", [128, Cout], bf16, kind="ExternalInput")
    if final_linear:
        t_wlinb = nc.dram_tensor("wlinb", [128, Cout], bf16, kind="ExternalInput")
        t_blin2 = nc.dram_tensor("blin2", [128, 1], f32, kind="ExternalInput")
        t_out = nc.dram_tensor("out", [DNP, 1], f32, kind="ExternalOutput")
        OC = 1
        odt = f32
    else:
        t_b1o = nc.dram_tensor("b1o", [128, Cout], bf16, kind="ExternalInput")
        t_out = nc.dram_tensor("h", [DNP, Cout], bf16, kind="ExternalOutput")
        OC = Cout
        odt = bf16
    t_eidx = nc.dram_tensor("eidx", [128, ICT], i16, kind="ExternalInput")
    t_dstl = nc.dram_tensor("dstl", [128, GCT], bf16, kind="ExternalInput")
    t_drow = nc.dram_tensor("drow", [1, GCT * P], bf16, kind="ExternalInput")
    tabL = nc.dram_tensor("tabL", [NPAD, ROW], bf16, kind="Internal")

    with tile.TileContext(nc) as tc:
        nc.gpsimd.load_library(library_config.mlp)
        with tc.tile_pool(name="const", bufs=1) as cpool:
            iota_row8 = cpool.tile([P, MAXG, P], bf16)
            nc.gpsimd.iota(iota_row8[:], pattern=[[0, MAXG], [1, P]], base=0,
                           channel_multiplier=0,
                           allow_small_or_imprecise_dtypes=True)
            iota_col4 = cpool.tile([P, 4, P], bf16)
            nc.gpsimd.iota(iota_col4[:], pattern=[[0, 4], [0, P]], base=0,
                           channel_multiplier=1,
                           allow_small_or_imprecise_dtypes=True)
            ones_bf = cpool.tile([1, P], bf16)
            nc.gpsimd.iota(ones_bf[:], pattern=[[0, P]], base=1,
                           channel_multiplier=0,
                           allow_small_or_imprecise_dtypes=True)
            attb = cpool.tile([P, Cout], bf16)
            nc.sync.dma_start(out=attb[:], in_=t_attb[:])
            attb8 = cpool.tile([P, MAXG, Cout], bf16)
            nc.vector.tensor_scalar(
                out=attb8[:],
                in0=attb[:, None, :].broadcast_to([P, MAXG, Cout]),
                scalar1=1.0, scalar2=None, op0=mybir.AluOpType.mult)
            wl = cpool.tile([Cin, Cout + 1], bf16)
            wr = cpool.tile([Cin, Cout], bf16)
            bl = cpool.tile([P, Cout + 1], bf16)
            br = cpool.tile([P, Cout], bf16)
            nc.sync.dma_start(out=wl[:], in_=t_wl[:])
            nc.sync.dma_start(out=wr[:], in_=t_wr[:])
            nc.sync.dma_start(out=bl[:], in_=t_bl[:])
            nc.sync.dma_start(out=br[:], in_=t_br[:])
            if final_linear:
                wlinb = cpool.tile([P, Cout], bf16)
                nc.sync.dma_start(out=wlinb[:], in_=t_wlinb[:])
                blin2 = cpool.tile([P, 1], f32)
                nc.sync.dma_start(out=blin2[:], in_=t_blin2[:])
            else:
                b1o = cpool.tile([P, Cout], bf16)
                nc.sync.dma_start(out=b1o[:], in_=t_b1o[:])
            tabR = cpool.tile([P, NW, Cout], bf16)

            # ---------------- node phase ----------------
            with tc.tile_pool(name="nsb", bufs=4) as npool, \
                 tc.tile_pool(name="nps", bufs=4, space="PSUM") as npsum:
                # xl table -> DRAM (gather source), rows [xl | 1]
                nt = NPAD // P
                blk = 0
                while blk < nt:
                    k = min(NB, nt - blk)
                    xt = npool.tile([Cin, NB * P], bf16, tag="xt", name="xt")
                    nc.sync.dma_start(out=xt[:, :k * P],
                                      in_=t_xT[:, blk * P:(blk + k) * P])
                    ot = npool.tile([P, NB, Cout + 1], bf16, tag="ot", name="ot")
                    for i0 in range(0, k, 4):
                        kk = min(4, k - i0)
                        ps = npsum.tile([P, 4, Cout + 1], f32, space="PSUM",
                                        tag="ps", name="ps")
                        for i in range(i0, i0 + kk):
                            nc.tensor.matmul(out=ps[:, i - i0, :],
                                             lhsT=xt[:, i * P:(i + 1) * P],
                                             rhs=wl[:], start=True, stop=True)
                        nc.vector.tensor_tensor(
                            out=ot[:, i0:i0 + kk, :], in0=ps[:, :kk, :],
                            in1=bl[:, None, :].broadcast_to([P, kk, Cout + 1]),
                            op=AL.add)
                    dv = tabL[blk * P:(blk + k) * P, 0:Cout + 1].rearrange(
                        "(b p) c -> p b c", p=P)
                    nc.sync.dma_start(out=dv, in_=ot[:, :k, :])
                    blk += k
                # xr table -> SBUF resident, per-window tiles
                blk = 0
                while blk < NW:
                    k = min(NB, NW - blk)
                    xt = npool.tile([Cin, NB * P], bf16, tag="xt", name="xt2")
                    nc.sync.dma_start(out=xt[:, :k * P],
                                      in_=t_xdT[:, blk * P:(blk + k) * P])
                    for i0 in range(0, k, 4):
                        kk = min(4, k - i0)
                        ps = npsum.tile([P, 4, Cout], f32, space="PSUM",
                                        tag="ps", name="ps2")
                        for i in range(i0, i0 + kk):
                            nc.tensor.matmul(out=ps[:, i - i0, :],
                                             lhsT=xt[:, i * P:(i + 1) * P],
                                             rhs=wr[:], start=True, stop=True)
                        nc.vector.tensor_tensor(
                            out=tabR[:, blk + i0:blk + i0 + kk, :],
                            in0=ps[:, :kk, :],
                            in1=br[:, None, :].broadcast_to([P, kk, Cout]),
                            op=AL.add)
                    blk += k

            tc.strict_bb_all_engine_barrier()

            # ---------------- edge phase ----------------
            with tc.tile_pool(name="esb", bufs=2) as ep, \
                 tc.tile_pool(name="exl", bufs=6) as xp, \
                 tc.tile_pool(name="eoh", bufs=8) as op_, \
                 tc.tile_pool(name="etmp", bufs=4) as tp, \
                 tc.tile_pool(name="eacc", bufs=2 * BW, space="PSUM") as aps, \
                 tc.tile_pool(name="ebc", bufs=2, space="PSUM") as bps, \
                 tc.tile_pool(name="exr", bufs=2, space="PSUM") as xps:
                qn = 0
                for L in layout:
                    w0, nw, GP = L["w0"], L["nw"], L["GP"]
                    icols = sum(8 * g for (_, _, g) in L["instrs"])
                    idxT = ep.tile([P, icols], i16, tag="idx", name="idx")
                    nc.sync.dma_start(
                        out=idxT[:],
                        in_=t_eidx[:, L["icol_off"]:L["icol_off"] + icols])
                    dstlT = ep.tile([P, GP], bf16, tag="dstl", name="dstl")
                    nc.scalar.dma_start(
                        out=dstlT[:],
                        in_=t_dstl[:, L["gcol_off"]:L["gcol_off"] + GP])
                    drow = ep.tile([1, GP * P], bf16, tag="drow", name="drow")
                    nc.scalar.dma_start(
                        out=drow[:],
                        in_=t_drow[:, L["gcol_off"] * P:(L["gcol_off"] + GP) * P])
                    eT = ep.tile([P, GP], f32, tag="e", name="e")
                    wT = ep.tile([P, GP], bf16, tag="w", name="w")
                    acc = [aps.tile([P, Cout + 1], f32, space="PSUM", tag="acc",
                                    name=f"acc{i}") for i in range(nw)]
                    gw = L["groups_w"]
                    first = [True] * nw
                    lastg = [max((g for g in range(GP) if gw[g] == w), default=-1)
                             for w in range(nw)]
                    outt = ep.tile([P, BW, OC], odt, tag="outt", name="outt")

                    ic = 0
                    for (c, goff, G) in L["instrs"]:
                        xl = xp.tile([P, MAXG, ROW], bf16, tag="xl", name="xl")
                        ni = G * P
                        nc.gpsimd.dma_gather(
                            xl[:, :G, :], tabL[c * CHUNK:, :],
                            idxT[:, ic:ic + 8 * G], ni, ni, ROW,
                            queue_num=qn, single_packet=False)
                        qn = (qn + 1) % 4
                        ic += 8 * G
                        g0 = goff
                        b = G
                        OT = op_.tile([P, MAXG, P], bf16, tag="OT", name="OT")
                        a4 = 0
                        while a4 < b:
                            b4 = min(4, b - a4)
                            pbc = bps.tile([P, 4 * P], f32, space="PSUM",
                                           tag="bc", name="bc")
                            nc.tensor.matmul(
                                out=pbc[:, :b4 * P], lhsT=ones_bf[:],
                                rhs=drow[0:1, (g0 + a4) * P:(g0 + a4 + b4) * P],
                                start=True, stop=True)
                            nc.vector.tensor_tensor(
                                out=OT[:, a4:a4 + b4, :],
                                in0=iota_col4[:, :b4, :],
                                in1=pbc[:, :b4 * P].rearrange(
                                    "p (g j) -> p g j", g=b4),
                                op=AL.is_equal)
                            a4 += b4
                        pxr = xps.tile([P, MAXG, Cout], f32, space="PSUM",
                                       tag="xr", name="xr")
                        for gi in range(b):
                            nc.tensor.matmul(
                                out=pxr[:, gi, :], lhsT=OT[:, gi, :],
                                rhs=tabR[:, w0 + gw[g0 + gi], :],
                                start=True, stop=True)
                        z = tp.tile([P, MAXG, Cout], bf16, tag="z", name="z")
                        nc.vector.tensor_tensor(
                            out=z[:, :b, :],
                            in0=xl[:, :b, 0:Cout],
                            in1=pxr[:, :b, :], op=AL.add)
                        zs = tp.tile([P, MAXG, Cout], bf16, tag="zs", name="zs")
                        nc.scalar.mul(zs[:, :b, :], z[:, :b, :], 0.2)
                        zl = tp.tile([P, MAXG, Cout], bf16, tag="zl", name="zl")
                        nc.vector.tensor_tensor(out=zl[:, :b, :],
                                                in0=z[:, :b, :],
                                                in1=zs[:, :b, :], op=AL.max)
                        em = tp.tile([P, MAXG, Cout], bf16, tag="em", name="em")
                        nc.vector.tensor_tensor(
                            out=em[:, :b, :], in0=zl[:, :b, :],
                            in1=attb8[:, :b, :], op=AL.mult)
                        nc.vector.tensor_reduce(
                            out=eT[:, g0:g0 + b], in_=em[:, :b, :],
                            axis=mybir.AxisListType.X, op=AL.add)
                        nc.scalar.activation(out=wT[:, g0:g0 + b],
                                             in_=eT[:, g0:g0 + b],
                                             func=AF.Exp)
                        O = op_.tile([P, MAXG, P], bf16, tag="O", name="O")
                        nc.vector.tensor_tensor(
                            out=O[:, :b, :],
                            in0=iota_row8[:, :b, :],
                            in1=dstlT[:, g0:g0 + b, None].broadcast_to(
                                [P, b, P]),
                            op=AL.is_equal)
                        xlw = tp.tile([P, MAXG, Cout + 1], bf16, tag="xlw",
                                      name="xlw")
                        nc.vector.tensor_tensor(
                            out=xlw[:, :b, :],
                            in0=xl[:, :b, 0:Cout + 1],
                            in1=wT[:, g0:g0 + b, None].broadcast_to(
                                [P, b, Cout + 1]),
                            op=AL.mult)
                        for gi in range(b):
                            w = gw[g0 + gi]
                            nc.tensor.matmul(out=acc[w][:],
                                             lhsT=O[:, gi, :],
                                             rhs=xlw[:, gi, :],
                                             start=first[w],
                                             stop=(g0 + gi == lastg[w]))
                            first[w] = False

                    for w in range(nw):
                        if lastg[w] < 0:
                            # no edges mapped here (dummy window): emit zeros
                            nc.vector.memset(outt[:, w, :], 0.0)
                            continue
                        r = tp.tile([P, 1], f32, tag="r", name="r")
                        nc.vector.reciprocal(r[:], acc[w][:, Cout:Cout + 1])
                        if final_linear:
                            v = tp.tile([P, Cout], f32, tag="v", name="v")
                            nc.vector.tensor_tensor(out=v[:],
                                                    in0=acc[w][:, :Cout],
                                                    in1=wlinb[:], op=AL.mult)
                            sv = tp.tile([P, 1], f32, tag="sv", name="sv")
                            nc.vector.tensor_reduce(out=sv[:], in_=v[:],
                                                    axis=mybir.AxisListType.X,
                                                    op=AL.add)
                            sv2 = tp.tile([P, 1], f32, tag="sv2", name="sv2")
                            nc.vector.tensor_tensor(out=sv2[:], in0=sv[:],
                                                    in1=r[:], op=AL.mult)
                            nc.vector.tensor_tensor(out=outt[:, w, :],
                                                    in0=sv2[:], in1=blin2[:],
                                                    op=AL.add)
                        else:
                            t1 = tp.tile([P, Cout], f32, tag="t1", name="t1")
                            nc.vector.tensor_tensor(
                                out=t1[:], in0=acc[w][:, :Cout],
                                in1=r[:, 0:1].broadcast_to([P, Cout]),
                                op=AL.mult)
                            t2 = tp.tile([P, Cout], f32, tag="t2", name="t2")
                            nc.vector.tensor_tensor(out=t2[:], in0=t1[:],
                                                    in1=b1o[:], op=AL.add)
                            t3 = tp.tile([P, Cout], f32, tag="t3", name="t3")
                            nc.scalar.mul(t3[:], t2[:], 0.01)
                            nc.vector.tensor_tensor(out=outt[:, w, :],
                                                    in0=t3[:], in1=t2[:],
                                                    op=AL.max)
                    ov = t_out[w0 * P:(w0 + nw) * P, :].rearrange(
                        "(b p) c -> p b c", p=P)
                    nc.sync.dma_start(out=ov, in_=outt[:, :nw, :])
    nc.compile()
    return nc


_CACHE = {}


def kernel(x, edge_index, W1l, b1l, W1r, b1r, att1, bias1,
           W2l, b2l, W2r, b2r, att2, bias2, Wlin, blin):
    from concourse import bass_utils

    x = np.asarray(x, np.float32)
    edge_index = np.asarray(edge_index)
    loops = np.arange(N, dtype=np.int64)
    src = np.concatenate([edge_index[0].astype(np.int64), loops])
    dst = np.concatenate([edge_index[1].astype(np.int64), loops])

    # degree-sorted relabeling (stable: ties keep id order)
    deg = np.bincount(dst, minlength=N)
    perm = np.argsort(-deg, kind="stable")          # new id -> old id
    newid = np.empty(NPAD, np.int64)
    newid[perm] = np.arange(N)
    src_p = newid[src]
    dst_p = newid[dst]

    layout, ICT, GCT, idx_all, dstl_all, dstlrow_all = _build_plan(src_p, dst_p)

    key = ("k2", ICT, GCT)
    if key not in _CACHE:
        _CACHE[key] = (
            _build_gat_layer(F, H1, layout, ICT, GCT, final_linear=False),
            _build_gat_layer(H1, H2, layout, ICT, GCT, final_linear=True),
        )
    ncA, ncB = _CACHE[key]

    def bcast(v, n=128):
        return np.tile(np.asarray(v, BF16)[None, :], (n, 1))

    # permuted, padded node features, transposed, bf16
    xP = np.zeros((NPAD, F), BF16)
    xP[:N] = x[perm].astype(BF16)
    xT = np.ascontiguousarray(xP.T)

    # per-device window -> global window map
    dev_wins = [[d + 8 * i for i in range(NW) if d + 8 * i < NPAD // P]
                for d in range(NDEV)]

    def dst_slice(xTfull):
        outs = []
        for d in range(NDEV):
            xd = np.zeros((xTfull.shape[0], DNP), BF16)
            for i, w in enumerate(dev_wins[d]):
                xd[:, i * P:(i + 1) * P] = xTfull[:, w * P:(w + 1) * P]
            outs.append(xd)
        return outs

    # ---- dispatch A (layer 1) ----
    wl1 = np.zeros((F, H1 + 1), BF16); wl1[:, :H1] = np.asarray(W1l, BF16)
    bl1 = np.zeros((128, H1 + 1), BF16)
    bl1[:, :H1] = np.asarray(b1l, BF16); bl1[:, H1] = 1.0
    xd_list = dst_slice(xT)
    in_maps = []
    for d in range(NDEV):
        in_maps.append(dict(
            xT=xT, xdT=xd_list[d], wl=wl1, bl=bl1,
            wr=np.asarray(W1r, BF16), br=bcast(b1r),
            attb=bcast(att1), b1o=bcast(bias1),
            eidx=idx_all[d], dstl=dstl_all[d], drow=dstlrow_all[d]))
    resA = bass_utils.run_bass_kernel_spmd(ncA, in_maps, core_ids=list(range(NDEV)))

    h1 = np.zeros((NPAD, H1), BF16)
    for d in range(NDEV):
        hd = resA.results[d]["h"]
        for i, w in enumerate(dev_wins[d]):
            h1[w * P:(w + 1) * P] = hd[i * P:(i + 1) * P]
    h1[N:] = 0
    h1T = np.ascontiguousarray(h1.T)

    # ---- dispatch B (layer 2 + head) ----
    wl2 = np.zeros((H1, H2 + 1), BF16); wl2[:, :H2] = np.asarray(W2l, BF16)
    bl2 = np.zeros((128, H2 + 1), BF16)
    bl2[:, :H2] = np.asarray(b2l, BF16); bl2[:, H2] = 1.0
    blin2 = float(np.asarray(bias2, np.float32) @ np.asarray(Wlin, np.float32).reshape(H2)
                  + np.asarray(blin, np.float32)[0])
    xd_list = dst_slice(h1T)
    in_maps = []
    for d in range(NDEV):
        in_maps.append(dict(
            xT=h1T, xdT=xd_list[d], wl=wl2, bl=bl2,
            wr=np.asarray(W2r, BF16), br=bcast(b2r),
            attb=bcast(att2),
            wlinb=np.tile(np.asarray(Wlin, BF16).reshape(1, H2), (128, 1)),
            blin2=np.full((128, 1), blin2, np.float32),
            eidx=idx_all[d], dstl=dstl_all[d], drow=dstlrow_all[d]))
    resB = bass_utils.run_bass_kernel_spmd(ncB, in_maps, core_ids=list(range(NDEV)))

    out_p = np.zeros(NPAD, np.float32)
    for d in range(NDEV):
        od = resB.results[d]["out"][:, 0]
        for i, w in enumerate(dev_wins[d]):
            out_p[w * P:(w + 1) * P] = od[i * P:(i + 1) * P]
    out = np.zeros(N, np.float32)
    out[perm] = out_p[:N]

    kernel._last_exec_ns = (resA.exec_time_ns, resB.exec_time_ns)
    kernel._last_results = (resA, resB)
    return out
